# revision 20
# baseline (speedup 1.0000x reference)
"""DARTS recurrent cell on Trainium2 (Bass/Tile), 8-core batch-sharded SPMD.

Math per timestep t (per batch row):
    ch0 = [x_t, h_prev] @ W0            (1024 -> 1024)
    s0  = h_prev + sigmoid(ch0[:512]) * (tanh(ch0[512:]) - h_prev)
    for i, (act, pred) in enumerate(GENOTYPE):
        ch = s_pred @ Ws[i]             (512 -> 1024)
        s_{i+1} = s_pred + sigmoid(ch[:512]) * (act(ch[512:]) - s_pred)
    h_t = mean(s_1..s_8)

Device layout: everything is kept transposed — hidden dim on SBUF
partitions (4 tiles of 128), batch (8 per core) on the free dim.  Weight
matrices are the stationary matmul operand (lhsT), the state streams as
the moving operand, so no on-chip transposes are ever needed.  Host code
pre-permutes x / weights into the exact SBUF tile layouts and inverts the
permutation on the output.
"""

import numpy as np

import concourse.bass as bass
import concourse.tile as tile
from concourse import bacc, mybir
from concourse.bass import ds
from concourse.bass_utils import run_bass_kernel_spmd

T, B, H = 1024, 64, 512
NCORES = 8
BL = B // NCORES          # batch rows per core
KH = H // 128             # 4  K-tiles per H
GENOTYPE = [("sigmoid", 0), ("relu", 1), ("relu", 1), ("identity", 1),
            ("tanh", 2), ("sigmoid", 5), ("tanh", 3), ("relu", 5)]

F32 = mybir.dt.float32
AF = mybir.ActivationFunctionType

# --- tunables -------------------------------------------------------------
SCHUNK = 8                # timesteps per For_i iteration (static inner unroll)
MM_DT = mybir.dt.bfloat16  # matmul operand dtype (weights + moving state)
USE_U = True              # precompute u = x_t @ W0[:H] once per chunk
COL_SPLIT = False         # split each weight tile into 2 column-groups
# --------------------------------------------------------------------------

NCHUNK = T // SCHUNK
SW = BL * KH              # 32 free-dim columns per (step) state tile


def _np_dt(dt):
    if dt == F32:
        return np.float32
    import ml_dtypes
    return ml_dtypes.bfloat16


def _u_ap(ups, s, half):
    """View into the per-chunk u PSUM tile [128, 8m*64] for step s.
    Column m*64 + s*BL + b holds uT[m*128+p, b] of that step; half 0 ->
    m 0..3 (c part), half 1 -> m 4..7 (h part).  Returns [128, 4, BL]."""
    v = ups[:].rearrange("p (m sb) -> p m sb", m=8)
    return v[:, half * 4:(half + 1) * 4, s * BL:(s + 1) * BL]


def _emit_step(nc, wk, pp, w0t, wst, hT, hTm, xt, ot, s, ups):
    """Emit one timestep.  hT: f32 state [128, SW]; hTm: matmul-dtype copy
    (same tile when MM_DT == F32).  xt: x chunk tile; ot: output chunk;
    ups: per-chunk u PSUM tile (USE_U) or None."""
    mixed = hTm is not hT

    def _mm_group(lhs_t, psum, lhs_base, rhs_of_k, nk):
        # accumulate over nk K-tiles for all 8 M-tiles
        for m in range(8):
            for k in range(nk):
                base = (lhs_base + k * 8 + m) * 128
                if COL_SPLIT:
                    for hf in range(2):
                        nc.tensor.matmul(
                            psum[hf * 64:(hf + 1) * 64,
                                 m * BL:(m + 1) * BL],
                            lhs_t[:, base + hf * 64:base + (hf + 1) * 64],
                            rhs_of_k(k),
                            start=(k == 0), stop=(k == nk - 1),
                            tile_position=(0, hf * 64),
                            skip_group_check=True,
                        )
                else:
                    nc.tensor.matmul(
                        psum[:, m * BL:(m + 1) * BL],
                        lhs_t[:, base:base + 128],
                        rhs_of_k(k),
                        start=(k == 0), stop=(k == nk - 1),
                    )

    # ---- ch0 = [x_t, h_prev] @ W0 ----
    ps0 = pp.tile([128, 8 * BL], F32, tag="ps")

    if ups is None:
        def rhs0(k):
            if k < KH:
                return xt[:, s * SW + k * BL:s * SW + (k + 1) * BL]
            return hTm[:, (k - KH) * BL:(k - KH + 1) * BL]

        _mm_group(w0t, ps0, 0, rhs0, 2 * KH)
        cin0, hin0 = ps0[:, :SW], ps0[:, SW:]
    else:
        # recurrent half only: h_prev @ W0[H:]; add precomputed u from PSUM
        _mm_group(w0t, ps0, 4 * 8,
                  lambda k: hTm[:, k * BL:(k + 1) * BL], KH)
        tc0 = wk.tile([128, SW], F32, tag="tc0")
        th0 = wk.tile([128, SW], F32, tag="th0")
        r3 = lambda ap: ap.rearrange("p (k b) -> p k b", k=KH)
        nc.vector.tensor_add(r3(tc0[:]), r3(ps0[:, :SW]), _u_ap(ups, s, 0))
        nc.vector.tensor_add(r3(th0[:]), r3(ps0[:, SW:]), _u_ap(ups, s, 1))
        cin0, hin0 = tc0[:], th0[:]

    c0 = wk.tile([128, SW], F32, tag="c")
    h0 = wk.tile([128, SW], F32, tag="h")
    nc.scalar.activation(c0[:], cin0, AF.Sigmoid)
    nc.scalar.activation(h0[:], hin0, AF.Tanh)
    d = wk.tile([128, SW], F32, tag="d")
    e = wk.tile([128, SW], F32, tag="e")
    s0 = wk.tile([128, SW], F32, tag="s0")
    nc.vector.tensor_sub(d[:], h0[:], hT[:])
    nc.vector.tensor_mul(e[:], d[:], c0[:])
    nc.vector.tensor_add(s0[:], e[:], hT[:])

    S = [s0]
    Sm = [None] * 9          # matmul-dtype copies (mixed precision only)

    def rhs_state(i):
        if not mixed:
            return S[i]
        if Sm[i] is None:
            sm = wk.tile([128, SW], MM_DT, tag=f"sm{i}")
            nc.vector.tensor_copy(sm[:], S[i][:])
            Sm[i] = sm
        return Sm[i]

    for i, (act, pred) in enumerate(GENOTYPE):
        ps = pp.tile([128, 8 * BL], F32, tag="ps")
        sp_m = rhs_state(pred)
        _mm_group(wst, ps, 32 * i, lambda k: sp_m[:, k * BL:(k + 1) * BL], KH)

        sp = S[pred]
        ci = wk.tile([128, SW], F32, tag="c")
        nc.scalar.activation(ci[:], ps[:, :SW], AF.Sigmoid)
        if act == "identity":
            hi = ps[:, SW:]
        elif act == "relu":
            hi = wk.tile([128, SW], F32, tag="h")
            nc.vector.tensor_scalar_max(hi[:], ps[:, SW:], 0.0)
            hi = hi[:]
        else:
            hi = wk.tile([128, SW], F32, tag="h")
            nc.scalar.activation(hi[:], ps[:, SW:],
                                 AF.Sigmoid if act == "sigmoid" else AF.Tanh)
            hi = hi[:]
        di = wk.tile([128, SW], F32, tag="d")
        ei = wk.tile([128, SW], F32, tag="e")
        si = wk.tile([128, SW], F32, tag=f"s{i + 1}")
        nc.vector.tensor_sub(di[:], hi, sp[:])
        nc.vector.tensor_mul(ei[:], di[:], ci[:])
        nc.vector.tensor_add(si[:], ei[:], sp[:])
        S.append(si)

    # ---- h_new = mean(S[1..8]) ----
    def tree_add(lo, hi):
        if hi == lo:
            return S[lo]
        mid = (lo + hi) // 2
        a = tree_add(lo, mid)
        b = tree_add(mid + 1, hi)
        r = wk.tile([128, SW], F32, tag=f"t{lo}_{hi}")
        nc.vector.tensor_add(r[:], a[:], b[:])
        return r

    acc = tree_add(1, 8)
    # state for next step (f32) + output chunk slice
    nc.scalar.activation(hT[:], acc[:], AF.Copy, scale=0.125)
    nc.vector.tensor_scalar_mul(ot[:, s * SW:(s + 1) * SW], acc[:], 0.125)
    if mixed:
        nc.scalar.activation(hTm[:], acc[:], AF.Copy, scale=0.125)


def build(nchunk=NCHUNK, schunk=SCHUNK, reps=1):
    nc = bacc.Bacc("TRN2")
    rows = nchunk * 128
    cols = schunk * SW
    x_d = nc.dram_tensor("x", [rows, cols], MM_DT, kind="ExternalInput")
    w0_d = nc.dram_tensor("w0", [128, 64 * 128], MM_DT, kind="ExternalInput")
    ws_d = nc.dram_tensor("ws", [128, 256 * 128], MM_DT, kind="ExternalInput")
    o_d = nc.dram_tensor("o", [rows, cols], F32, kind="ExternalOutput")

    with tile.TileContext(nc) as tc:
        with (
            tc.tile_pool(name="wts", bufs=1) as wp,
            tc.tile_pool(name="state", bufs=1) as stp,
            tc.tile_pool(name="xin", bufs=2) as xp,
            tc.tile_pool(name="oout", bufs=2) as op,
            tc.tile_pool(name="wk", bufs=3) as wk,
            tc.tile_pool(name="ps", bufs=7 if USE_U else 8, space="PSUM") as pp,
            tc.tile_pool(name="psu", bufs=1, space="PSUM") as ppu,
        ):
            w0t = wp.tile([128, 64 * 128], MM_DT)
            nc.gpsimd.dma_start(w0t[:], w0_d[:])
            wst = wp.tile([128, 256 * 128], MM_DT)
            nc.gpsimd.dma_start(wst[:], ws_d[:])

            hT = stp.tile([128, SW], F32)
            nc.vector.memset(hT[:], 0.0)
            if MM_DT == F32:
                hTm = hT
            else:
                hTm = stp.tile([128, SW], MM_DT)
                nc.vector.memset(hTm[:], 0.0)

            import contextlib

            with contextlib.ExitStack() as lstk:
                if reps > 1:
                    lstk.enter_context(tc.For_i(0, reps, 1))
                cb = lstk.enter_context(
                    tc.For_i(0, rows, 128,
                             hint_engines=(mybir.EngineType.PE,
                                           mybir.EngineType.DVE)))
                xt = xp.tile([128, cols], MM_DT)
                nc.gpsimd.dma_start(xt[:], x_d[ds(cb, 128), :])
                ot = op.tile([128, cols], F32)
                us = None
                if USE_U:
                    # uT for all schunk steps: x @ W0[:H] -> [128, 8m x
                    # (schunk*BL)] in one PSUM bank, then one copy to SBUF.
                    upt = ppu.tile([128, 8 * schunk * BL], F32, tag="u")
                    for m in range(8):
                        for k in range(KH):
                            rhs = xt[:].rearrange(
                                "p (t k b) -> p k t b", k=KH, b=BL)[:, k]
                            nc.tensor.matmul(
                                upt[:, m * schunk * BL:(m + 1) * schunk * BL],
                                w0t[:, (k * 8 + m) * 128:(k * 8 + m + 1) * 128],
                                rhs,
                                start=(k == 0), stop=(k == KH - 1),
                            )
                    us = xp.tile([128, 8 * schunk * BL], F32, tag="usb")
                    nc.scalar.copy(us[:], upt[:])
                for s in range(schunk):
                    _emit_step(nc, wk, pp, w0t, wst, hT, hTm, xt, ot, s, us)
                nc.gpsimd.dma_start(o_d[ds(cb, 128), :], ot[:])

    nc.finalize()
    return nc


# ---------------------------------------------------------------------------
# host-side packing


def pack_x(x_local, nchunk=NCHUNK, schunk=SCHUNK):
    """[T', BL, H] -> [nchunk*128, schunk*SW] in tile layout
    [c*128+p, s*SW + k*BL + b] = x[c*schunk+s, b, k*128+p]."""
    tt = nchunk * schunk
    a = x_local[:tt].reshape(nchunk, schunk, BL, KH, 128)
    a = a.transpose(0, 4, 1, 3, 2)           # c p s k b
    return np.ascontiguousarray(
        a.reshape(nchunk * 128, schunk * SW)).astype(_np_dt(MM_DT))


def unpack_o(o, nchunk=NCHUNK, schunk=SCHUNK):
    a = o.reshape(nchunk, 128, schunk, KH, BL)
    a = a.transpose(0, 2, 4, 3, 1)           # c s b k p
    return np.ascontiguousarray(a.reshape(nchunk * schunk, BL, H))


def pack_w0(W0):
    a = W0.reshape(8, 128, 8, 128).transpose(1, 0, 2, 3)   # p k m c
    return np.ascontiguousarray(a.reshape(128, 64 * 128)).astype(_np_dt(MM_DT))


def pack_ws(Ws):
    a = Ws.reshape(8, KH, 128, 8, 128).transpose(2, 0, 1, 3, 4)  # p i k m c
    return np.ascontiguousarray(a.reshape(128, 256 * 128)).astype(_np_dt(MM_DT))


_CACHE = {}


def kernel(inputs, W0, Ws):
    inputs = np.asarray(inputs, dtype=np.float32)
    W0 = np.asarray(W0, dtype=np.float32)
    Ws = np.asarray(Ws, dtype=np.float32)

    if "nc" not in _CACHE:
        _CACHE["nc"] = build()
    nc = _CACHE["nc"]

    w0t = pack_w0(W0)
    wst = pack_ws(Ws)
    in_maps = []
    for j in range(NCORES):
        xj = pack_x(inputs[:, j * BL:(j + 1) * BL, :])
        in_maps.append({"x": xj, "w0": w0t, "ws": wst})

    res = run_bass_kernel_spmd(nc, in_maps, core_ids=list(range(NCORES)))
    _CACHE["last_res"] = res

    hiddens = np.empty((T, B, H), dtype=np.float32)
    for j in range(NCORES):
        hiddens[:, j * BL:(j + 1) * BL, :] = unpack_o(res.results[j]["o"])
    return hiddens, hiddens[-1][None]
